# revision 2
# baseline (speedup 1.0000x reference)
"""Trainium2 Bass kernel for nn_CyclicShuffle: grouped 1x1 conv with activation/weight
quantization, BN (inference) + ReLU + residual.

Strategy: data-parallel over batch (64 batches -> 8 per core), fp16 I/O.
The 2e-2 rel-err budget is ~45x the fp16 cost (measured 9.0e-3), so x is fed to
the device as fp16 and y is returned as fp16, halving the HBM traffic that
dominates this kernel (fp32: 51.4MB/core ~ 143us floor; fp16: 25.7MB ~ 71us).

Per core:
  - Quantized activations are stored as 128+q (q in 0..15) in bf16: the cast
    to bf16 rounds at ulp=1 in [128,256), so one DVE op (max 128, min 143)
    performs round+clip in the cast. The 128 offset is a per-output-channel
    constant (128*rowsum(w_int)) folded into the BN bias.
  - Weights quantized on-chip to odd integers -15..15 in bf16 (exact); grouped
    conv runs exactly on the PE with fp32 PSUM accumulation; 1/225 folded into
    the BN scale.
  - Elementwise passes balanced across engines: quant pass1 (s*x + b+128 ->
    bf16) on Pool(x2)/ACT(x1)/DVE(x1) per batch; pass2 round+clip on DVE;
    BN+ReLU (fp16 out) on ACT from PSUM; residual add (fp16) on DVE.
  - Channel blocks in pairs of 128 (one group = one 256-wide pair): [128,2*784]
    fp16 tiles, 1568B DMA descriptors.
Self-contained: shapes hardcoded, no sibling imports.
"""

import numpy as np

B, C, HW = 64, 1024, 784          # x: [64, 1024, 28, 28] fp32 at the interface
G, CG = 4, 256
N_CORES = 8
BPC = B // N_CORES                 # batches per core
NBLK = C // 128                    # 8 channel blocks of 128
NPAIR = 4                          # pairs of blocks = groups
HW2 = 2 * HW                       # 1568
NHALF = HW // 2                    # 392 (psum bank limit is 512 fp32)
MAGIC = float(2.0 ** 23)
OFF = 128.0                        # bf16 magic offset: 128+q exact, ulp=1

_COMPILED = None


def _build_program(s_a, b_a, s_w, neg_lw, eps, repeats=1):
    """Build the SPMD Bass/Tile program. Scalar quant params are baked as immediates.
    repeats>1 duplicates the main loop (same I/O) for slope-based timing."""
    from contextlib import ExitStack
    from concourse import bacc, bass, tile, masks, mybir

    f32 = mybir.dt.float32
    f16 = mybir.dt.float16
    bf16 = mybir.dt.bfloat16
    AF = mybir.ActivationFunctionType
    OP = mybir.AluOpType
    AX = mybir.AxisListType

    nc = bacc.Bacc("TRN2", target_bir_lowering=False, debug=False)

    x_d = nc.dram_tensor("x", [BPC, C, HW], f16, kind="ExternalInput")
    w_d = nc.dram_tensor("w", [C, CG], f32, kind="ExternalInput")
    gamma_d = nc.dram_tensor("gamma", [C], f32, kind="ExternalInput")
    beta_d = nc.dram_tensor("beta", [C], f32, kind="ExternalInput")
    mean_d = nc.dram_tensor("mean", [C], f32, kind="ExternalInput")
    var_d = nc.dram_tensor("var", [C], f32, kind="ExternalInput")
    y_d = nc.dram_tensor("y", [BPC, C, HW], f16, kind="ExternalOutput")

    with tile.TileContext(nc) as tc, ExitStack() as ctx:
        const = ctx.enter_context(tc.tile_pool(name="const", bufs=1))
        wpool = ctx.enter_context(tc.tile_pool(name="wprep", bufs=1))
        xpool = ctx.enter_context(tc.tile_pool(name="x", bufs=12))
        tpool = ctx.enter_context(tc.tile_pool(name="t", bufs=6))
        apool = ctx.enter_context(tc.tile_pool(name="a", bufs=8))
        rpool = ctx.enter_context(tc.tile_pool(name="r", bufs=8))
        pspool = ctx.enter_context(tc.tile_pool(name="ps", bufs=6, space="PSUM"))
        wtps = ctx.enter_context(tc.tile_pool(name="wtps", bufs=2, space="PSUM"))

        # ---------- one-time: weight quantization + transpose ----------
        ident = const.tile([128, 128], bf16)
        masks.make_identity(nc, ident[:])
        ident_f32 = const.tile([128, 128], f32)
        masks.make_identity(nc, ident_f32[:])

        wq = wpool.tile([128, NBLK * CG], f32)     # wq[p, blk*256+k] = w[blk*128+p, k]
        for blk in range(NBLK):
            nc.gpsimd.dma_start(out=wq[:, blk * CG:(blk + 1) * CG],
                                in_=w_d[blk * 128:(blk + 1) * 128, :])
        # u = (w - lW) * s_w  (two roundings, matches ref div-then-mul up to 1ulp)
        nc.vector.tensor_scalar(out=wq[:], in0=wq[:], scalar1=neg_lw, scalar2=s_w,
                                op0=OP.add, op1=OP.mult)
        # shifted round + upper clip
        nc.vector.tensor_scalar(out=wq[:], in0=wq[:], scalar1=MAGIC,
                                scalar2=MAGIC + 15.0, op0=OP.add, op1=OP.min)
        # lower clip + unshift -> q in 0..15 (2M+15 is NOT fp32-representable, so
        # unshift before the affine)
        wint = wpool.tile([128, NBLK * CG], bf16)
        nc.vector.tensor_scalar(out=wq[:], in0=wq[:], scalar1=MAGIC, scalar2=-MAGIC,
                                op0=OP.max, op1=OP.add)
        # w_int = 2q - 15 (odd integers -15..15, exact in bf16)
        nc.vector.tensor_scalar(out=wint[:], in0=wq[:], scalar1=2.0, scalar2=-15.0,
                                op0=OP.mult, op1=OP.add)
        # rowsum[p, blk] = sum_k w_int[blk*128+p, k]  (exact ints, fp32 out)
        rs_t = const.tile([128, NBLK], f32)
        for blk in range(NBLK):
            nc.vector.reduce_sum(out=rs_t[:, blk:blk + 1],
                                 in_=wint[:, blk * CG:(blk + 1) * CG], axis=AX.X)
        # transpose the 16 [128,128] chunks: WT[:, (j*2+kc)*128+m] = wint[m, j*256+kc*128+:]
        wt = const.tile([128, 16 * 128], bf16)
        for j in range(NBLK):
            for kc in range(2):
                pst = wtps.tile([128, 128], bf16, name=f"pst{j}_{kc}", tag="pst", bufs=1)
                nc.tensor.transpose(pst[:], wint[:, j * CG + kc * 128: j * CG + (kc + 1) * 128],
                                    ident[:])
                nc.vector.tensor_copy(wt[:, (j * 2 + kc) * 128:(j * 2 + kc + 1) * 128], pst[:])

        # ---------- one-time: BN fold ----------
        # S[p, j] = gamma/(225*sqrt(var+eps)) for channel c = j*128+p ; Bc = beta - mean*inv
        # Load [1024] params as contiguous [8,128] (8 fat descriptors, not 1024
        # element-strided ones), then PE-transpose to the [128,8] layout.
        def load_param(dram, nm):
            t8 = const.tile([8, 128], f32, name=nm + "8", tag=nm + "8")
            nc.gpsimd.dma_start(out=t8[:], in_=dram.ap().rearrange("(a p) -> a p", p=128))
            pt = wtps.tile([128, 8], f32, name=nm + "ps", tag="parps", bufs=1)
            nc.tensor.transpose(pt[:], t8[:], ident_f32[:8, :8])
            t = const.tile([128, NBLK], f32, name=nm, tag=nm)
            nc.vector.tensor_copy(t[:], pt[:])
            return t

        g_t = load_param(gamma_d, "g_t")
        b_t = load_param(beta_d, "b_t")
        m_t = load_param(mean_d, "m_t")
        v_t = load_param(var_d, "v_t")
        eps_t = const.tile([128, 1], f32)
        nc.gpsimd.memset(eps_t[:], float(50625.0 * eps))
        sq = const.tile([128, NBLK], f32)
        nc.scalar.activation(sq[:], v_t[:], AF.Sqrt, scale=50625.0, bias=eps_t[:])
        rec = const.tile([128, NBLK], f32)
        nc.vector.reciprocal(rec[:], sq[:])
        s_t = const.tile([128, NBLK], f32)
        nc.vector.tensor_tensor(out=s_t[:], in0=g_t[:], in1=rec[:], op=OP.mult)
        bc_t = const.tile([128, NBLK], f32)
        nc.vector.scalar_tensor_tensor(out=bc_t[:], in0=m_t[:], scalar=-225.0, in1=s_t[:],
                                       op0=OP.mult, op1=OP.mult)
        nc.vector.tensor_tensor(out=bc_t[:], in0=bc_t[:], in1=b_t[:], op=OP.add)
        # fold the 128-offset of the activations: bc2 = bc - 128*rowsum*s
        rss = const.tile([128, NBLK], f32)
        nc.vector.tensor_tensor(out=rss[:], in0=rs_t[:], in1=s_t[:], op=OP.mult)
        nc.vector.scalar_tensor_tensor(out=bc_t[:], in0=rss[:], scalar=-OFF, in1=bc_t[:],
                                       op0=OP.mult, op1=OP.add)

        # ---------- main loop ----------
        # pair q holds channel blocks 2q, 2q+1 (= group q's 256 channels) laid out
        # [128, k*784 + hw] with channel = q*256 + k*128 + p
        for b in [bb for _ in range(repeats) for bb in range(BPC)]:
            xt = []
            at = []
            for q in range(NPAIR):
                xq = xpool.tile([128, 2, HW], f16, tag="x", name=f"x{b}_{q}")
                nc.sync.dma_start(
                    out=xq[:],
                    in_=x_d[b, q * CG:(q + 1) * CG, :].rearrange("(k p) hw -> p k hw", p=128))
                xt.append(xq)
            for q in range(NPAIR):
                # pass1: t = s_a*x + (b_a+128); bf16 output cast rounds at ulp=1
                # in [128,256) => integer rounding for in-range values.
                tq = tpool.tile([128, 2, HW], bf16, tag="t", name=f"t{b}_{q}")
                if q < 2:      # Pool engine
                    nc.gpsimd.tensor_scalar(out=tq[:], in0=xt[q][:], scalar1=float(s_a),
                                            scalar2=float(b_a + OFF), op0=OP.mult, op1=OP.add)
                elif q == 2:   # ACT engine (free affine via Copy)
                    nc.scalar.activation(tq[:], xt[q][:], AF.Copy, scale=float(s_a),
                                         bias=float(b_a + OFF))
                else:          # DVE
                    nc.vector.tensor_scalar(out=tq[:], in0=xt[q][:], scalar1=float(s_a),
                                            scalar2=float(b_a + OFF), op0=OP.mult, op1=OP.add)
                # pass2 (DVE, 2-byte 2x): clip to [128, 143] = 128 + q, q in 0..15
                aq = apool.tile([128, 2, HW], bf16, tag="a", name=f"a{b}_{q}")
                nc.vector.tensor_scalar(out=aq[:], in0=tq[:], scalar1=OFF,
                                        scalar2=OFF + 15.0, op0=OP.max, op1=OP.min)
                at.append(aq)
            for g in range(G):
                src = (g + 1) % NPAIR   # source pair (shuffle): group g reads group g+1
                for oc in range(2):
                    j = 2 * g + oc
                    ps = [pspool.tile([128, NHALF], f32, tag="ps", name=f"ps{b}_{j}_{h}")
                          for h in range(2)]
                    for kc in range(2):
                        lhsT = wt[:, (j * 2 + kc) * 128:(j * 2 + kc + 1) * 128]
                        for half in range(2):
                            nc.tensor.matmul(
                                ps[half][:], lhsT,
                                at[src][:, kc, half * NHALF:(half + 1) * NHALF],
                                start=(kc == 0), stop=(kc == 1))
                    for half in range(2):
                        rt = rpool.tile([128, NHALF], f16, tag="r", name=f"r{b}_{j}_{half}")
                        nc.scalar.activation(rt[:], ps[half][:], AF.Relu,
                                             scale=s_t[:, j:j + 1], bias=bc_t[:, j:j + 1])
                        sl = xt[g][:, oc, half * NHALF:(half + 1) * NHALF]
                        nc.vector.tensor_tensor(out=sl, in0=rt[:], in1=sl, op=OP.add)
                nc.scalar.dma_start(
                    out=y_d[b, g * CG:(g + 1) * CG, :].rearrange("(k p) hw -> p k hw", p=128),
                    in_=xt[g][:])

    nc.compile()
    return nc


def kernel(x, weight, lW, uW, lA, uA, gamma, beta, running_mean, running_var):
    global _COMPILED
    from concourse.bass_utils import run_bass_kernel_spmd

    x = np.asarray(x, dtype=np.float32).reshape(B, C, HW).astype(np.float16)
    weight = np.ascontiguousarray(np.asarray(weight, dtype=np.float32))
    lW = np.float32(lW); uW = np.float32(uW); lA = np.float32(lA); uA = np.float32(uA)
    gamma = np.ascontiguousarray(np.asarray(gamma, dtype=np.float32))
    beta = np.ascontiguousarray(np.asarray(beta, dtype=np.float32))
    mean = np.ascontiguousarray(np.asarray(running_mean, dtype=np.float32))
    var = np.ascontiguousarray(np.asarray(running_var, dtype=np.float32))

    s_a = np.float32(15.0) / (uA - lA)
    b_a = -lA * s_a
    s_w = np.float32(15.0) / np.float32(uW - lW)

    key = (float(s_a), float(b_a), float(s_w), float(-lW))
    if _COMPILED is None or _COMPILED[0] != key:
        nc = _build_program(float(s_a), float(b_a), float(s_w), float(-lW), 1e-5)
        _COMPILED = (key, nc)
    nc = _COMPILED[1]

    in_maps = []
    for c in range(N_CORES):
        in_maps.append({
            "x": x[c * BPC:(c + 1) * BPC],
            "w": weight,
            "gamma": gamma, "beta": beta, "mean": mean, "var": var,
        })
    res = run_bass_kernel_spmd(nc, in_maps, list(range(N_CORES)))
    out = np.concatenate([res.results[c]["y"] for c in range(N_CORES)], axis=0)
    return out.reshape(B, C, 28, 28).astype(np.float32)


# revision 20
# speedup vs baseline: 1.7833x; 1.7833x over previous
"""Trainium2 Bass kernel for nn_CyclicShuffle: grouped 1x1 conv with activation/weight
quantization, BN (inference) + ReLU + residual.

Strategy: data-parallel over batch (64 batches -> 8 per core), fp16 I/O.
The 2e-2 rel-err budget is ~45x the fp16 cost (measured 9.0e-3), so x is fed to
the device as fp16 and y is returned as fp16, halving the HBM traffic that
dominates this kernel (fp32: 51.4MB/core ~ 143us floor; fp16: 25.7MB ~ 71us).

Per core:
  - Quantized activations are stored as 128+q (q in 0..15) in bf16: the cast
    to bf16 rounds at ulp=1 in [128,256), so one DVE op (max 128, min 143)
    performs round+clip in the cast. The 128 offset is a per-output-channel
    constant (128*rowsum(w_int)) folded into the BN bias.
  - Weights quantized on-chip to odd integers -15..15 in bf16 (exact); grouped
    conv runs exactly on the PE with fp32 PSUM accumulation; 1/225 folded into
    the BN scale.
  - Elementwise passes balanced across engines: quant pass1 (s*x + b+128 ->
    bf16, the cast does the rounding) on ACT(x2)/DVE(x2) per batch; pass2
    clip on DVE; BN+ReLU (fp16 out) on ACT from PSUM into a per-group staging
    tile; ONE merged residual add per group (fp16, 2-byte DVE) in-place into x.
  - Software pipelining: batch b+1's DMA-in + quant ops are emitted BEFORE
    batch b's conv/residual groups. Engines drain queues in program order, so
    this prevents DVE head-of-line blocking (idle through the conv phase);
    measured 135us -> 117us per 8-batch pass.
  - Channel blocks in pairs of 128 (one group = one 256-wide pair): [128,2*784]
    fp16 tiles, 1568B DMA descriptors.
  - Timing variants (repeats>1) wrap the body in a tc.For_i hardware loop so
    device exec dominates the axon dispatch floor at large R.
Self-contained: shapes hardcoded, no sibling imports.
"""

import numpy as np

B, C, HW = 64, 1024, 784          # x: [64, 1024, 28, 28] fp32 at the interface
G, CG = 4, 256
N_CORES = 8
BPC = B // N_CORES                 # batches per core
NBLK = C // 128                    # 8 channel blocks of 128
NPAIR = 4                          # pairs of blocks = groups
HW2 = 2 * HW                       # 1568
NHALF = HW // 2                    # 392 (psum bank limit is 512 fp32)
MAGIC = float(2.0 ** 23)
OFF = 128.0                        # bf16 magic offset: 128+q exact, ulp=1

_COMPILED = None


def _build_program(s_a, b_a, s_w, neg_lw, eps, repeats=1,
                   pass1_eng=("act", "dve", "act", "dve"),
                   in_pk=False, out_pk=False, io_f32=False,
                   skip_p2=False, skip_res=False, skip_p1=False,
                   merged_res=True):
    """Build the SPMD Bass/Tile program. Scalar quant params are baked as immediates.
    repeats>1 duplicates the main loop (same I/O) for slope-based timing.
    pass1_eng: engine per group-pair for the quant affine pass.
    in_pk/out_pk: TIMING-ONLY probes - use channel-pair interleaved "(p k) hw"
    APs (3136B descriptors) for input/output DMA. Numerically WRONG (weights
    not permuted to match); only for descriptor-granularity benchmarks.
    io_f32: TIMING-ONLY probe - fp32 x/y dram tensors (baseline I/O width)."""
    from contextlib import ExitStack
    from concourse import bacc, bass, tile, masks, mybir

    f32 = mybir.dt.float32
    f16 = mybir.dt.float32 if io_f32 else mybir.dt.float16
    bf16 = mybir.dt.bfloat16
    AF = mybir.ActivationFunctionType
    OP = mybir.AluOpType
    AX = mybir.AxisListType

    nc = bacc.Bacc("TRN2", target_bir_lowering=False, debug=False)

    x_d = nc.dram_tensor("x", [BPC, C, HW], f16, kind="ExternalInput")
    w_d = nc.dram_tensor("w", [C, CG], f32, kind="ExternalInput")
    gamma_d = nc.dram_tensor("gamma", [C], f32, kind="ExternalInput")
    beta_d = nc.dram_tensor("beta", [C], f32, kind="ExternalInput")
    mean_d = nc.dram_tensor("mean", [C], f32, kind="ExternalInput")
    var_d = nc.dram_tensor("var", [C], f32, kind="ExternalInput")
    y_d = nc.dram_tensor("y", [BPC, C, HW], f16, kind="ExternalOutput")

    with tile.TileContext(nc) as tc, ExitStack() as ctx:
        const = ctx.enter_context(tc.tile_pool(name="const", bufs=1))
        wpool = ctx.enter_context(tc.tile_pool(name="wprep", bufs=1))
        xpool = ctx.enter_context(tc.tile_pool(name="x", bufs=12))
        tpool = ctx.enter_context(tc.tile_pool(name="t", bufs=6))
        apool = ctx.enter_context(tc.tile_pool(name="a", bufs=8))
        rpool = ctx.enter_context(tc.tile_pool(name="r", bufs=8))
        pspool = ctx.enter_context(tc.tile_pool(name="ps", bufs=6, space="PSUM"))
        wtps = ctx.enter_context(tc.tile_pool(name="wtps", bufs=2, space="PSUM"))

        # ---------- one-time: weight quantization + transpose ----------
        ident = const.tile([128, 128], bf16)
        masks.make_identity(nc, ident[:])
        ident_f32 = const.tile([128, 128], f32)
        masks.make_identity(nc, ident_f32[:])

        wq = wpool.tile([128, NBLK * CG], f32)     # wq[p, blk*256+k] = w[blk*128+p, k]
        for blk in range(NBLK):
            nc.gpsimd.dma_start(out=wq[:, blk * CG:(blk + 1) * CG],
                                in_=w_d[blk * 128:(blk + 1) * 128, :])
        # u = (w - lW) * s_w  (two roundings, matches ref div-then-mul up to 1ulp)
        nc.vector.tensor_scalar(out=wq[:], in0=wq[:], scalar1=neg_lw, scalar2=s_w,
                                op0=OP.add, op1=OP.mult)
        # shifted round + upper clip
        nc.vector.tensor_scalar(out=wq[:], in0=wq[:], scalar1=MAGIC,
                                scalar2=MAGIC + 15.0, op0=OP.add, op1=OP.min)
        # lower clip + unshift -> q in 0..15 (2M+15 is NOT fp32-representable, so
        # unshift before the affine)
        wint = wpool.tile([128, NBLK * CG], bf16)
        nc.vector.tensor_scalar(out=wq[:], in0=wq[:], scalar1=MAGIC, scalar2=-MAGIC,
                                op0=OP.max, op1=OP.add)
        # w_int = 2q - 15 (odd integers -15..15, exact in bf16)
        nc.vector.tensor_scalar(out=wint[:], in0=wq[:], scalar1=2.0, scalar2=-15.0,
                                op0=OP.mult, op1=OP.add)
        # rowsum[p, blk] = sum_k w_int[blk*128+p, k]  (exact ints, fp32 out)
        rs_t = const.tile([128, NBLK], f32)
        for blk in range(NBLK):
            nc.vector.reduce_sum(out=rs_t[:, blk:blk + 1],
                                 in_=wint[:, blk * CG:(blk + 1) * CG], axis=AX.X)
        # transpose the 16 [128,128] chunks: WT[:, (j*2+kc)*128+m] = wint[m, j*256+kc*128+:]
        wt = const.tile([128, 16 * 128], bf16)
        for j in range(NBLK):
            for kc in range(2):
                pst = wtps.tile([128, 128], bf16, name=f"pst{j}_{kc}", tag="pst", bufs=1)
                nc.tensor.transpose(pst[:], wint[:, j * CG + kc * 128: j * CG + (kc + 1) * 128],
                                    ident[:])
                nc.vector.tensor_copy(wt[:, (j * 2 + kc) * 128:(j * 2 + kc + 1) * 128], pst[:])

        # ---------- one-time: BN fold ----------
        # S[p, j] = gamma/(225*sqrt(var+eps)) for channel c = j*128+p ; Bc = beta - mean*inv
        # Load [1024] params as contiguous [8,128] (8 fat descriptors, not 1024
        # element-strided ones), then PE-transpose to the [128,8] layout.
        def load_param(dram, nm):
            t8 = const.tile([8, 128], f32, name=nm + "8", tag=nm + "8")
            nc.gpsimd.dma_start(out=t8[:], in_=dram.ap().rearrange("(a p) -> a p", p=128))
            pt = wtps.tile([128, 8], f32, name=nm + "ps", tag="parps", bufs=1)
            nc.tensor.transpose(pt[:], t8[:], ident_f32[:8, :8])
            t = const.tile([128, NBLK], f32, name=nm, tag=nm)
            nc.vector.tensor_copy(t[:], pt[:])
            return t

        g_t = load_param(gamma_d, "g_t")
        b_t = load_param(beta_d, "b_t")
        m_t = load_param(mean_d, "m_t")
        v_t = load_param(var_d, "v_t")
        eps_t = const.tile([128, 1], f32)
        nc.gpsimd.memset(eps_t[:], float(50625.0 * eps))
        sq = const.tile([128, NBLK], f32)
        nc.scalar.activation(sq[:], v_t[:], AF.Sqrt, scale=50625.0, bias=eps_t[:])
        rec = const.tile([128, NBLK], f32)
        nc.vector.reciprocal(rec[:], sq[:])
        s_t = const.tile([128, NBLK], f32)
        nc.vector.tensor_tensor(out=s_t[:], in0=g_t[:], in1=rec[:], op=OP.mult)
        bc_t = const.tile([128, NBLK], f32)
        nc.vector.scalar_tensor_tensor(out=bc_t[:], in0=m_t[:], scalar=-225.0, in1=s_t[:],
                                       op0=OP.mult, op1=OP.mult)
        nc.vector.tensor_tensor(out=bc_t[:], in0=bc_t[:], in1=b_t[:], op=OP.add)
        # fold the 128-offset of the activations: bc2 = bc - 128*rowsum*s
        rss = const.tile([128, NBLK], f32)
        nc.vector.tensor_tensor(out=rss[:], in0=rs_t[:], in1=s_t[:], op=OP.mult)
        nc.vector.scalar_tensor_tensor(out=bc_t[:], in0=rss[:], scalar=-OFF, in1=bc_t[:],
                                       op0=OP.mult, op1=OP.add)

        # ---------- main loop ----------
        # pair q holds channel blocks 2q, 2q+1 (= group q's 256 channels) laid out
        # [128, k*784 + hw] with channel = q*256 + k*128 + p
        # repeats>1: hardware loop (For_i) re-runs the whole 8-batch body on
        # device with identical I/O -- tiny NEFF, so wall time at large R is
        # dominated by true device exec, not axon dispatch.
        loop_ctx = tc.For_i(0, repeats) if repeats > 1 else None
        if loop_ctx is not None:
            loop_ctx.__enter__()

        # Software pipelining: emit batch b+1's DMA-in + quant BEFORE batch b's
        # conv/residual groups. Engines drain their queues in program order, so
        # without this, DVE head-of-line-blocks on batch b's residual adds and
        # idles through the conv phase instead of pre-quantizing b+1 (measured
        # as near-additive DVE cost: 138us vs 96us skeleton).
        def emit_load_quant(b):
            xt, at = [], []
            for q in range(NPAIR):
                xq = xpool.tile([128, 2, HW], f16, tag="x", name=f"x{b}_{q}")
                rearr = "(p k) hw -> p k hw" if in_pk else "(k p) hw -> p k hw"
                nc.sync.dma_start(
                    out=xq[:],
                    in_=x_d[b, q * CG:(q + 1) * CG, :].rearrange(rearr, p=128))
                xt.append(xq)
            for q in range(NPAIR):
                if skip_p1:    # TIMING-ONLY ablation: PE reads raw fp16 x
                    at.append(xt[q])
                    continue
                # pass1: t = s_a*x + (b_a+128); bf16 output cast rounds at ulp=1
                # in [128,256) => integer rounding for in-range values.
                tq = tpool.tile([128, 2, HW], bf16, tag="t", name=f"t{b}_{q}")
                eng = pass1_eng[q]
                if eng == "pool":
                    nc.gpsimd.tensor_scalar(out=tq[:], in0=xt[q][:], scalar1=float(s_a),
                                            scalar2=float(b_a + OFF), op0=OP.mult, op1=OP.add)
                elif eng == "act":
                    nc.scalar.activation(tq[:], xt[q][:], AF.Copy, scale=float(s_a),
                                         bias=float(b_a + OFF))
                else:
                    nc.vector.tensor_scalar(out=tq[:], in0=xt[q][:], scalar1=float(s_a),
                                            scalar2=float(b_a + OFF), op0=OP.mult, op1=OP.add)
                # pass2 (DVE, 2-byte 2x): clip to [128, 143] = 128 + q, q in 0..15
                if skip_p2:    # TIMING-ONLY ablation: feed unclipped tq to PE
                    at.append(tq)
                else:
                    aq = apool.tile([128, 2, HW], bf16, tag="a", name=f"a{b}_{q}")
                    nc.vector.tensor_scalar(out=aq[:], in0=tq[:], scalar1=OFF,
                                            scalar2=OFF + 15.0, op0=OP.max, op1=OP.min)
                    at.append(aq)
            return xt, at

        staged = {}

        def emit_conv_res(b, xt, at):
            for g in range(G):
                src = (g + 1) % NPAIR   # source pair (shuffle): group g reads group g+1
                rg = (rpool.tile([128, 2, HW], f16, tag="r", name=f"rg{b}_{g}")
                      if merged_res else None)
                for oc in range(2):
                    j = 2 * g + oc
                    ps = [pspool.tile([128, NHALF], f32, tag="ps", name=f"ps{b}_{j}_{h}")
                          for h in range(2)]
                    for kc in range(2):
                        lhsT = wt[:, (j * 2 + kc) * 128:(j * 2 + kc + 1) * 128]
                        for half in range(2):
                            nc.tensor.matmul(
                                ps[half][:], lhsT,
                                at[src][:, kc, half * NHALF:(half + 1) * NHALF],
                                start=(kc == 0), stop=(kc == 1))
                    for half in range(2):
                        if merged_res:
                            rt = rg[:, oc, half * NHALF:(half + 1) * NHALF]
                        else:
                            rt = rpool.tile([128, NHALF], f16, tag="r",
                                            name=f"r{b}_{j}_{half}")[:]
                        nc.scalar.activation(rt, ps[half][:], AF.Relu,
                                             scale=s_t[:, j:j + 1], bias=bc_t[:, j:j + 1])
                        if not merged_res and not skip_res:  # skip_res: TIMING-ONLY
                            sl = xt[g][:, oc, half * NHALF:(half + 1) * NHALF]
                            nc.vector.tensor_tensor(out=sl, in0=rt, in1=sl, op=OP.add)
                if merged_res:
                    # one big 2-byte DVE add per group (residual), in-place into x
                    nc.vector.tensor_tensor(out=xt[g][:], in0=rg[:], in1=xt[g][:],
                                            op=OP.add)
                rearro = "(p k) hw -> p k hw" if out_pk else "(k p) hw -> p k hw"
                nc.scalar.dma_start(
                    out=y_d[b, g * CG:(g + 1) * CG, :].rearrange(rearro, p=128),
                    in_=xt[g][:])

        staged[0] = emit_load_quant(0)
        for b in range(BPC):
            if b + 1 < BPC:
                staged[b + 1] = emit_load_quant(b + 1)
            xt, at = staged.pop(b)
            emit_conv_res(b, xt, at)
        if loop_ctx is not None:
            loop_ctx.__exit__(None, None, None)

    nc.compile()
    return nc


def kernel(x, weight, lW, uW, lA, uA, gamma, beta, running_mean, running_var):
    global _COMPILED
    from concourse.bass_utils import run_bass_kernel_spmd

    x = np.asarray(x, dtype=np.float32).reshape(B, C, HW).astype(np.float16)
    weight = np.ascontiguousarray(np.asarray(weight, dtype=np.float32))
    lW = np.float32(lW); uW = np.float32(uW); lA = np.float32(lA); uA = np.float32(uA)
    gamma = np.ascontiguousarray(np.asarray(gamma, dtype=np.float32))
    beta = np.ascontiguousarray(np.asarray(beta, dtype=np.float32))
    mean = np.ascontiguousarray(np.asarray(running_mean, dtype=np.float32))
    var = np.ascontiguousarray(np.asarray(running_var, dtype=np.float32))

    s_a = np.float32(15.0) / (uA - lA)
    b_a = -lA * s_a
    s_w = np.float32(15.0) / np.float32(uW - lW)

    key = (float(s_a), float(b_a), float(s_w), float(-lW))
    if _COMPILED is None or _COMPILED[0] != key:
        nc = _build_program(float(s_a), float(b_a), float(s_w), float(-lW), 1e-5)
        _COMPILED = (key, nc)
    nc = _COMPILED[1]

    in_maps = []
    for c in range(N_CORES):
        in_maps.append({
            "x": x[c * BPC:(c + 1) * BPC],
            "w": weight,
            "gamma": gamma, "beta": beta, "mean": mean, "var": var,
        })
    res = run_bass_kernel_spmd(nc, in_maps, list(range(N_CORES)))
    out = np.concatenate([res.results[c]["y"] for c in range(N_CORES)], axis=0)
    return out.reshape(B, C, 28, 28).astype(np.float32)
